# revision 1
# baseline (speedup 1.0000x reference)
"""CWTConvNet Trainium2 kernel.

The reference computes a 112-filter Morlet-wavelet SAME conv over length-2048
signals, then indexes the result with IMG_SELECT = linspace(0, 71, 224) cast
to int64 — i.e. only conv output positions 0..71 survive, each repeated 1-4
times. For those 72 positions only filter taps k in [209, 561) can touch
nonzero (non-pad) input, so the whole module reduces exactly to

    out72[f, s, l] = sum_{j=0}^{351} w2[f, j] * xe[s, j + l],   l in [0, 72)

with w2 = w_real[:, 0, 209:561] and xe = [71 zeros, x[s, 0:352], pad], then an
index-repeat expansion 72 -> 224 along the last axis.

Device kernel (per core, pure data parallel over 4 of 32 batches = 48
signals): the host supplies xe with groups of TI=24 signals interleaved
element-wise, so each im2col DMA descriptor carries 24 signals (3456B
contiguous runs — the im2col is descriptor/issue-limited otherwise). Each
group is a pipeline chain: 3 im2col DMAs (one per 128-tap contraction chunk,
all on the sync HWDGE ring so completions are FIFO), 4x3 accumulating
matmuls into 4 PSUM banks, plain PSUM->SBUF drains alternating between the
scalar and vector engines, and one store per bank. The store keeps the
(l, k)-interleaved PSUM column order; the host undoes the interleave,
applies the IMG_SELECT repeat-gather, and unshards — all in one numpy
pass.
"""

import numpy as np

import concourse.bacc as bacc
import concourse.bass as bass
import concourse.mybir as mybir
import concourse.tile as tile
from concourse.bass_utils import run_bass_kernel_spmd

# Problem constants (hardcoded; kernel.py must be self-contained).
B, C, L = 32, 12, 2048
F, K = 112, 561
NCORES = 8
BPC = B // NCORES          # batches per core
S = BPC * C                # signals per core (48)
NL = 72                    # conv output positions actually used
NI = 224                   # expanded output length
J = 352                    # taps that can touch non-pad input: k in [209, 561)
KOFF = 209                 # first needed tap
NCHUNK = 3                 # contraction chunks of 128 (352 -> 128,128,96)
XE_LEN = 456               # 71 zeros + 352 signal + tail zeros (>= 2*128+127+71+1)
XE_ZLEAD = 71

TI = 24                    # signals interleaved per im2col descriptor
NG = S // TI               # signal groups / pipeline chains per core (2)
NCOL_G = TI * NL           # matmul columns per group (1728)
NBANK = 4                  # PSUM banks per group (1728 fp32 cols)
NCOL_B = NCOL_G // NBANK   # columns per bank / matmul (432)
LPB = NL // NBANK          # l-positions per bank (18)

# Config: input dtype for the matmul operands. fp32 is exact; bf16 halves
# im2col DMA bytes and matmul passes at ~2e-3 relative error.
USE_BF16 = True

SEL = np.linspace(0, 71, NI, dtype=np.int64)

_CACHE = {}


def _build_nc():
    f32 = mybir.dt.float32
    dt_in = mybir.dt.bfloat16 if USE_BF16 else f32
    nc = bacc.Bacc("TRN2", target_bir_lowering=False, debug=False)

    # xg[g, t, k] = xe[TI*g + k, t]  (TI-signal element interleave)
    xg_d = nc.declare_dram_parameter("xg", [NG, XE_LEN * TI], dt_in, isOutput=False)
    w_d = nc.declare_dram_parameter("w2t", [128, NCHUNK, F], dt_in, isOutput=False)
    # y[f, g, (l k)] keeps the interleaved PSUM column order; host undoes it.
    y_d = nc.declare_dram_parameter("y", [F, NG, NCOL_G], f32, isOutput=True)

    with tile.TileContext(nc) as tc:
        with (
            tc.tile_pool(name="sbuf", bufs=1) as pool,
            tc.tile_pool(name="psum", bufs=1, space="PSUM") as psum_pool,
        ):
            w_t = pool.tile([128, NCHUNK, F], dt_in, tag="w", name="w")
            nc.scalar.dma_start(out=w_t[:], in_=w_d.ap())

            psum_u = [
                psum_pool.tile([128, NCOL_B], f32, tag=f"ps{u}", name=f"ps{u}")
                for u in range(NG * NBANK)
            ]

            # im2col: rhs[p, (l k)] = xg[g, (128jc + p + l)*TI + k].
            # All on the sync ring: same-ring DMAs complete FIFO, so group 0's
            # chunks land first and its chain starts while later groups stream.
            # Chunk jc covers taps [128jc, 128jc + kr) with kr < 128 for the
            # last chunk (352 taps total) — no need to move or multiply the
            # zero-padded tail rows.
            krows = [min(128, J - 128 * jc) for jc in range(NCHUNK)]
            rhs = {}
            for g in range(NG):
                for jc in range(NCHUNK):
                    kr = krows[jc]
                    r_t = pool.tile(
                        [128, NCOL_G], dt_in,
                        tag=f"rhs{g}_{jc}", name=f"rhs{g}_{jc}",
                    )
                    # The very last chunk gates the kernel tail: split it into
                    # bank-pair column halves so the first two banks' final
                    # matmuls/drain/store overlap the second half's stream.
                    if g == NG - 1 and jc == NCHUNK - 1:
                        half = NCOL_G // 2
                        for hh in range(2):
                            src = bass.AP(
                                tensor=xg_d,
                                offset=g * XE_LEN * TI + 128 * jc * TI + hh * half,
                                ap=[[TI, kr], [1, half]],
                            )
                            nc.sync.dma_start(
                                out=r_t[:kr, hh * half : (hh + 1) * half], in_=src
                            )
                    else:
                        src = bass.AP(
                            tensor=xg_d,
                            offset=g * XE_LEN * TI + 128 * jc * TI,
                            ap=[[TI, kr], [1, NCOL_G]],
                        )
                        nc.sync.dma_start(out=r_t[:kr], in_=src)
                    rhs[(g, jc)] = r_t

            for g in range(NG):
                for jc in range(NCHUNK):
                    kr = krows[jc]
                    for b in range(NBANK):
                        nc.tensor.matmul(
                            psum_u[g * NBANK + b][:F, :],
                            w_t[:kr, jc, :],
                            rhs[(g, jc)][:kr, b * NCOL_B : (b + 1) * NCOL_B],
                            start=(jc == 0),
                            stop=(jc == NCHUNK - 1),
                        )
                # Plain contiguous drains (no de-interleave — host handles it)
                # on both PSUM-capable engines, then one store per PSUM bank.
                o72 = pool.tile([128, NCOL_G], f32, tag=f"o72_{g}", name=f"o72_{g}")
                for b in range(NBANK):
                    dst = o72[:F, b * NCOL_B : (b + 1) * NCOL_B]
                    if (g + b) % 2 == 0:
                        nc.scalar.copy(dst, psum_u[g * NBANK + b][:F, :])
                    else:
                        nc.vector.tensor_copy(out=dst, in_=psum_u[g * NBANK + b][:F, :])
                    if b % 2 == 1:
                        # One store per bank pair: fewer ~0.6us ring issues
                        # in the kernel tail.
                        nc.sync.dma_start(
                            out=y_d.ap()[:, g, (b - 1) * NCOL_B : (b + 1) * NCOL_B],
                            in_=o72[:F, (b - 1) * NCOL_B : (b + 1) * NCOL_B],
                        )

    nc.compile()
    return nc


def _get_nc():
    if "nc" not in _CACHE:
        _CACHE["nc"] = _build_nc()
    return _CACHE["nc"]


def _prepare_in_maps(x, w_real):
    if USE_BF16:
        import ml_dtypes

        np_in = np.dtype(ml_dtypes.bfloat16)
    else:
        np_in = np.dtype(np.float32)
    x = np.ascontiguousarray(np.asarray(x), dtype=np.float32)
    w_real = np.asarray(w_real, dtype=np.float32)

    w2t = np.zeros((NCHUNK * 128, F), np.float32)
    w2t[:J] = w_real[:, 0, KOFF:K].T
    w2t_dev = np.ascontiguousarray(
        w2t.reshape(NCHUNK, 128, F).transpose(1, 0, 2)
    ).astype(np_in)

    in_maps = []
    for m in range(NCORES):
        xe = np.zeros((S, XE_LEN), np.float32)
        xe[:, XE_ZLEAD : XE_ZLEAD + J] = x[m * BPC : (m + 1) * BPC].reshape(
            S, L
        )[:, :J]
        # interleave: xg[g, t, k] = xe[TI*g + k, t]
        xg = np.ascontiguousarray(
            xe.reshape(NG, TI, XE_LEN).transpose(0, 2, 1)
        ).reshape(NG, XE_LEN * TI)
        in_maps.append({"xg": xg.astype(np_in), "w2t": w2t_dev})
    return in_maps


def _assemble(results):
    # Device output: y[f, g, (l k)] with bank-major l split:
    # y[f, g, NCOL_B*b + TI*lo + k] = out72[f, TI*g + k, LPB*b + lo].
    ydev = np.stack([r["y"] for r in results])          # [8, F, NG, NCOL_G]
    yv = ydev.reshape(NCORES, F, NG, NBANK, LPB, TI)
    y72 = yv.transpose(0, 2, 5, 1, 3, 4)                # [8, NG, TI, F, NBANK, LPB]
    y72 = y72.reshape(NCORES, S, F, NL)                 # s = TI*g + k, l = LPB*b + lo
    y = y72[..., SEL]                                   # [8, S, F, NI]
    return np.ascontiguousarray(y.reshape(B, C, F, NI))


def kernel(x, w_real):
    nc = _get_nc()
    in_maps = _prepare_in_maps(x, w_real)
    res = run_bass_kernel_spmd(nc, in_maps, list(range(NCORES)))
    return _assemble(res.results)



# revision 2
# speedup vs baseline: 1.3228x; 1.3228x over previous
"""CWTConvNet Trainium2 kernel — flipped (weight-Toeplitz) formulation.

The reference reduces exactly (see previous baseline) to

    out72[f, s, l] = sum_{j=0}^{351} w2[f, j] * xe[s, j + l],  l in [0, 72)

with w2 = w_real[:, 0, 209:561] and xe = [71 zeros, x[s, 0:352], pad], then an
index-repeat expansion 72 -> 224 (IMG_SELECT) on the host.

Instead of im2col-ing the DATA (2.43 MB of HBM reads per core), this kernel
im2cols the WEIGHTS, which are shared by all 48 signals on a core:

    l = 9*lb + l',  lb in [0,8), l' in [0,9)
    OUT[(s,lb), (f,l')] = sum_c sum_p  xe[s, 128c + p + 9*lb] * w2[f, 128c + p - l']

Per pass of 16 signals, the stationary operand is the (tiny, host-built)
data tile stat[c][p, (s,lb)] = xe[s, 128c+p+9lb] (128x128, full PE array),
and the moving rhs is the weight-Toeplitz wt[c][p, (f,l')] = w2[f, 128c+p-l'],
which is signal-independent. Filter support lets chunks 1/2 carry only
filters 48..111 / 104..111 (and chunk 2 only rows p < 104), so the whole
weight bank is just 1008+576+72 columns. Per-core HBM traffic drops from
~4.1 MB (baseline) to ~1.5 MB: 718 KB of loads + 774 KB of bf16 stores.

A short burst of zero matmuls into a scratch PSUM bank warms the PE HAM
clock gate while the loads stream, so the real matmuls run at 2.4 GHz.
"""

import numpy as np

import concourse.bacc as bacc
import concourse.bass as bass
import concourse.mybir as mybir
import concourse.tile as tile
from concourse.bass_utils import run_bass_kernel_spmd

# Problem constants (hardcoded; kernel.py must be self-contained).
B, C, L = 32, 12, 2048
F, K = 112, 561
KOFF = 209                 # first needed tap; w2 = w_real[:, 0, 209:561]
J = 352                    # taps per filter window
NCORES = 8
BPC = B // NCORES          # batches per core
S = BPC * C                # signals per core (48)
NL = 72                    # conv output positions actually used
NI = 224                   # expanded output length

NT = 3                     # passes per core (16 signals each)
SG = 16                    # signals per pass
NLB, NLP = 8, 9            # l = NLP*lb + l'; SG*NLB = 128 partitions exactly
F1LO, F2LO = 48, 104       # first filter with support in chunks 1 / 2
NC0 = F * NLP              # 1008 weight cols, chunk 0
NC1 = (F - F1LO) * NLP     # 576,  chunk 1 (filters 48..111)
NC2 = (F - F2LO) * NLP     # 72,   chunk 2 (filters 104..111)
C2ROWS = 104               # chunk 2 rows p >= 104 are all-zero weights
XE_LEN = 448               # 71 zeros + 352 signal + tail (max index 446)
XE_ZLEAD = 71

NDUMMY = 6                 # PE warm-up matmuls (zeros) while loads stream

SEL = np.linspace(0, 71, NI, dtype=np.int64)

_CACHE = {}


def _build_nc():
    f32 = mybir.dt.float32
    bf16 = mybir.dt.bfloat16
    nc = bacc.Bacc("TRN2", target_bir_lowering=False, debug=False)

    # packA: stat(0,0) | wt0            -> pass-0 chunk-0 matmuls start early
    # packB: stat(0,1) | stat(0,2) | wt1 | wt2
    # packC: stat(1,0..2) | stat(2,0..2)
    packA_d = nc.declare_dram_parameter("packA", [128, 128 + NC0], bf16, isOutput=False)
    packB_d = nc.declare_dram_parameter("packB", [128, 256 + NC1 + NC2], bf16, isOutput=False)
    packC_d = nc.declare_dram_parameter("packC", [128, 6 * 128], bf16, isOutput=False)
    y_d = nc.declare_dram_parameter("y", [NT, 128, NC0], bf16, isOutput=True)

    with tile.TileContext(nc) as tc:
        with (
            tc.tile_pool(name="sbuf", bufs=1) as pool,
            tc.tile_pool(name="psum", bufs=1, space="PSUM") as psum_pool,
        ):
            # Loads first so the sync engine issues them the moment its
            # preamble ends.
            bigA = pool.tile([128, 128 + NC0], bf16, tag="bigA", name="bigA")
            bigB = pool.tile([128, 256 + NC1 + NC2], bf16, tag="bigB", name="bigB")
            bigC = pool.tile([128, 6 * 128], bf16, tag="bigC", name="bigC")
            nc.sync.dma_start(out=bigA[:], in_=packA_d.ap())
            nc.sync.dma_start(out=bigB[:], in_=packB_d.ap())
            nc.sync.dma_start(out=bigC[:], in_=packC_d.ap())

            # PE HAM warm-up: zero matmuls into a scratch bank while the
            # loads stream. Never read back.
            scratch = pool.tile([128, 504], bf16, tag="scr", name="scr")
            nc.vector.memset(scratch[:], 0)
            ps_dummy = psum_pool.tile([128, 504], f32, tag="psD", name="psD")
            for _ in range(NDUMMY):
                nc.tensor.matmul(
                    ps_dummy[:, :], scratch[:, 0:128], scratch[:, :],
                    start=True, stop=True,
                )

            wt0 = bigA[:, 128 : 128 + NC0]
            wt1 = bigB[:, 256 : 256 + NC1]
            wt2 = bigB[:, 256 + NC1 : 256 + NC1 + NC2]
            stat = {
                (0, 0): bigA[:, 0:128],
                (0, 1): bigB[:, 0:128],
                (0, 2): bigB[:, 128:256],
            }
            for t in (1, 2):
                for c in range(3):
                    off = ((t - 1) * 3 + c) * 128
                    stat[(t, c)] = bigC[:, off : off + 128]

            for t in range(NT):
                psA = psum_pool.tile([128, 504], f32, tag=f"psA{t}", name=f"psA{t}")
                psB = psum_pool.tile([128, 504], f32, tag=f"psB{t}", name=f"psB{t}")
                st0, st1, st2 = stat[(t, 0)], stat[(t, 1)], stat[(t, 2)]
                # chunk 0 (all filters), stationary st0
                nc.tensor.matmul(psA[:, :], st0, wt0[:, 0:504], start=True, stop=False)
                nc.tensor.matmul(psB[:, :], st0, wt0[:, 504:NC0], start=True, stop=False)
                # chunk 1 (filters 48..111), stationary st1
                nc.tensor.matmul(psA[:, 432:504], st1, wt1[:, 0:72], start=False, stop=True)
                nc.tensor.matmul(psB[:, :], st1, wt1[:, 72:NC1], start=False, stop=False)
                # chunk 2 (filters 104..111, rows < 104), stationary st2
                nc.tensor.matmul(
                    psB[:, 432:504], st2[0:C2ROWS, :], wt2[0:C2ROWS, :],
                    start=False, stop=True,
                )
                # Drain both banks (fp32 -> bf16) on the two PSUM-capable
                # engines in parallel, then one store per pass on the
                # scalar HWDGE ring (loads own the sync ring).
                o = pool.tile([128, NC0], bf16, tag=f"o{t}", name=f"o{t}")
                nc.scalar.copy(o[:, 0:504], psA[:, :])
                nc.vector.tensor_copy(out=o[:, 504:NC0], in_=psB[:, :])
                nc.scalar.dma_start(out=y_d.ap()[t], in_=o[:])

    nc.compile()
    return nc


def _get_nc():
    if "nc" not in _CACHE:
        _CACHE["nc"] = _build_nc()
    return _CACHE["nc"]


def _build_wt(w2pad):
    """wt[c][p, (f-flo)*9+l'] = w2[f, 128c + p - l'] (zero outside [0, J))."""
    out = []
    for c, flo, rows in ((0, 0, 128), (1, F1LO, 128), (2, F2LO, C2ROWS)):
        p = np.arange(128)[:, None, None]
        f = np.arange(flo, F)[None, :, None]
        lp = np.arange(NLP)[None, None, :]
        j = 128 * c + p - lp
        val = np.where((j >= 0) & (j < J), w2pad[f, np.clip(j, 0, J - 1)], 0.0)
        val[rows:] = 0.0
        out.append(val.reshape(128, (F - flo) * NLP).astype(np.float32))
    return out


def _prepare_in_maps(x, w_real):
    import ml_dtypes

    np_bf16 = np.dtype(ml_dtypes.bfloat16)
    x = np.ascontiguousarray(np.asarray(x), dtype=np.float32)
    w_real = np.asarray(w_real, dtype=np.float32)

    w2 = w_real[:, 0, KOFF:K]                       # [112, 352]
    wt0, wt1, wt2 = _build_wt(w2)

    # Stationary index grid: q[c][p, sl, lb] = 128c + p + 9lb
    p = np.arange(128)[:, None, None]
    lb = np.arange(NLB)[None, None, :]
    qs = [128 * c + p + NLP * lb for c in range(3)]  # each [128, 1, 8]

    in_maps = []
    for m in range(NCORES):
        xc = x[m * BPC : (m + 1) * BPC].reshape(S, L)
        xe = np.zeros((S, XE_LEN), np.float32)
        xe[:, XE_ZLEAD : XE_ZLEAD + J] = xc[:, :J]
        st = {}
        for t in range(NT):
            sig = xe[SG * t : SG * (t + 1)]          # [16, 448]
            for c in range(3):
                # [128, 16, 8] -> [128, 128] with col = sl*8 + lb
                v = sig[np.arange(SG)[None, :, None], qs[c]]
                st[(t, c)] = v.reshape(128, SG * NLB)
        packA = np.concatenate([st[(0, 0)], wt0], axis=1)
        packB = np.concatenate([st[(0, 1)], st[(0, 2)], wt1, wt2], axis=1)
        packC = np.concatenate(
            [st[(t, c)] for t in (1, 2) for c in range(3)], axis=1
        )
        in_maps.append({
            "packA": np.ascontiguousarray(packA).astype(np_bf16),
            "packB": np.ascontiguousarray(packB).astype(np_bf16),
            "packC": np.ascontiguousarray(packC).astype(np_bf16),
        })
    return in_maps


def _assemble(results):
    # Device output y[t, sl*8+lb, f*9+l'] = out72[f, 16t+sl, 9lb+l'].
    ydev = np.stack([np.asarray(r["y"], dtype=np.float32) for r in results])
    yv = ydev.reshape(NCORES, NT, SG, NLB, F, NLP)
    out72 = yv.transpose(0, 1, 2, 4, 3, 5).reshape(NCORES, S, F, NL)
    y = out72[..., SEL]                              # [8, S, F, NI]
    return np.ascontiguousarray(y.reshape(B, C, F, NI), dtype=np.float32)


def kernel(x, w_real):
    nc = _get_nc()
    in_maps = _prepare_in_maps(x, w_real)
    res = run_bass_kernel_spmd(nc, in_maps, list(range(NCORES)))
    return _assemble(res.results)
